# revision 33
# baseline (speedup 1.0000x reference)
"""Multi-head attention (RoPE-by-head variant) on 8 TRN2 NeuronCores.

Sharding: tensor-parallel over heads. Core c owns heads [4c, 4c+4):
  - computes q/k/v projections for its 512 features,
  - causal attention for its 4 heads entirely in SBUF,
  - AllGather of per-core attention outputs (bf16, [512, S] each -> [4096, S]),
  - output projection for its 512 OUTPUT columns (no reduce needed),
  - host concatenates column slices and adds bo.

RoPE here is indexed by HEAD (not position) in the reference, so it is a
fixed per-head 2x2 rotation of feature pairs == a linear map folded into
Wq/Wk (and bq/bk) on the host, exactly. The 1/sqrt(HD) score scale is
folded into Wq as well.

Softmax skips max-subtraction: scores are ~N(0, 1.64) so |score| < 40 with
overwhelming margin; exp() in fp32 is safe and matches softmax exactly in
exact arithmetic. Masked entries get -1e30 -> exp == 0.

Schedule (v3): projections and attention interleave at half-sequence
granularity so the AllGather chain starts at ~30% of the kernel. The
v-projection emits the natural layout directly (x-tile stationary) so no
transposes are needed; q/k stay feature-major for the score matmuls.
Scores for two key tiles share one [128,512] psum tile and one exp.
Out-projection chunks are woven between the second burst's attention
pairs to keep the PE busy (and the HAM clock-gate warm) while exp paces
attention. AllGather SBUF reloads ride the gpsimd queue, pinned behind
late compute so the Tile scheduler cannot hoist their collective-waits
into earlier engine streams (its collective cost model is optimistic).
"""

import math
from contextlib import ExitStack

import ml_dtypes
import numpy as np

import concourse.bass as bass
import concourse.mybir as mybir
import concourse.tile as tile
from concourse import bacc, bass_utils
from concourse.masks import make_identity
from concourse.tile_rust import add_dep_helper

# Problem dims (hardcoded per contract).
B, S, D, H, HD = 1, 2048, 4096, 32, 128
NCORES = 8
HPC = H // NCORES          # heads per core = 4
FPC = HPC * HD             # features per core = 512
ROPE_BASE = 10000.0
P = 128                    # partitions

BF16 = mybir.dt.bfloat16
FP32 = mybir.dt.float32


# ---------------------------------------------------------------- builder --

def build_nc(s=S, hpc=HPC, ncores=NCORES, compute_dt=BF16):
    """Build the SPMD Bass program (identical on all cores; data differs)."""
    fpc = hpc * HD
    d = ncores * fpc                 # model dim (square weights)
    kc_n = d // P                    # contraction chunks for projections
    nq = s // P                      # 128-row query tiles (16)
    sh_w = s // 2                    # half width (interleave granularity)
    n_fc = fpc // P                  # feature chunks per core (4)
    VS = HD + 1                      # vp slot stride

    # AllGather chunks (query-column ranges); boundaries at pair ends (256).
    chunk_ws = [512, 512, 512, 256, 256]
    assert sum(chunk_ws) == s
    cum = list(np.cumsum(chunk_ws))
    n_ag = len(chunk_ws)

    nc = bacc.Bacc(
        "TRN2", target_bir_lowering=False, debug=False, num_devices=ncores
    )

    # Inputs (bf16 unless noted)
    qT = nc.dram_tensor("qT", [d, s], compute_dt, kind="ExternalInput")
    kT = nc.dram_tensor("kT", [d, s], compute_dt, kind="ExternalInput")
    vT = nc.dram_tensor("vT", [d, s], compute_dt, kind="ExternalInput")
    wqT = nc.dram_tensor("wqT", [d, fpc], compute_dt, kind="ExternalInput")
    wkT = nc.dram_tensor("wkT", [d, fpc], compute_dt, kind="ExternalInput")
    wvT = nc.dram_tensor("wvT", [d, fpc], compute_dt, kind="ExternalInput")
    woT = nc.dram_tensor("woT", [d, fpc], compute_dt, kind="ExternalInput")
    # per-partition biases for q/k proj, one column per (proj, f-chunk)
    bqkv = nc.dram_tensor("bqkv", [P, 3 * n_fc], FP32, kind="ExternalInput")
    # v bias as a row (natural layout needs it on the free axis)
    bvr = nc.dram_tensor("bvr", [1, fpc], compute_dt, kind="ExternalInput")
    # wide causal mask for the final key-tile pair: [diag | 0 | full | diag]
    maskw = nc.dram_tensor("maskw", [P, 4 * P], FP32, kind="ExternalInput")
    # Output: transposed slice yT = (out columns [c*fpc,(c+1)*fpc)).T
    yT = nc.dram_tensor("yT", [fpc, s], FP32, kind="ExternalOutput")

    with tile.TileContext(nc) as tc, ExitStack() as ctx:
        const = ctx.enter_context(tc.tile_pool(name="const", bufs=1))
        persist = ctx.enter_context(tc.tile_pool(name="persist", bufs=1))

        mask_sb = const.tile([P, 4 * P], FP32)
        nc.sync.dma_start(out=mask_sb, in_=maskw[:, :])
        bias_sb = const.tile([P, 3 * n_fc], FP32)
        nc.sync.dma_start(out=bias_sb, in_=bqkv[:, :])
        bvr_sb = const.tile([1, fpc], compute_dt)
        nc.sync.dma_start(out=bvr_sb, in_=bvr[:, :])
        ones_row = const.tile([1, P], compute_dt)
        nc.vector.memset(ones_row, 1.0)
        ident = const.tile([P, P], compute_dt)
        make_identity(nc, ident)

        # Persistent SBUF tensors
        qpT = [persist.tile([P, s], compute_dt, name=f"qpT{f}") for f in range(n_fc)]
        kpT = [persist.tile([P, s], compute_dt, name=f"kpT{f}") for f in range(n_fc)]
        # attention output, natural layout [sq, HD] blocks per query tile
        attnN = [persist.tile([P, nq * HD], compute_dt, name=f"attnN{h}")
                 for h in range(hpc)]
        # vp: natural layout per head; slot layout [v (HD) | 1.0 | pad] --
        # the ones column makes PV's matmul also produce the softmax
        # denominator.
        vp = [persist.tile([P, nq * VS], compute_dt, name=f"vp{h}")
              for h in range(hpc)]
        for h in range(hpc):
            ones_col = vp[h].rearrange("p (t c) -> p t c", c=VS)[:, :, HD:HD + 1]
            nc.vector.memset(ones_col, 1.0)
        # out-proj weights, prefetched during half-1 projections
        wo_sb = [persist.tile([P, fpc], compute_dt, name=f"wo{kc}")
                 for kc in range(kc_n)]

        dram_pool = ctx.enter_context(
            tc.tile_pool(name="dram", bufs=1, space="DRAM"))
        ag_in = [dram_pool.tile([fpc, chunk_ws[x]], compute_dt,
                                name=f"ag_in{x}") for x in range(n_ag)]
        ag_out = [dram_pool.tile([ncores * fpc, chunk_ws[x]], compute_dt,
                                 name=f"ag_out{x}", addr_space="Shared")
                  for x in range(n_ag)]

        # streaming pools (SBUF) live for the whole program
        xw = ctx.enter_context(tc.tile_pool(name="xw", bufs=6))
        probs_pool = ctx.enter_context(tc.tile_pool(name="probs", bufs=4))
        small = ctx.enter_context(tc.tile_pool(name="small", bufs=4))
        attnT_pool = ctx.enter_context(tc.tile_pool(name="attnT", bufs=2))
        ag_sb_pool = ctx.enter_context(tc.tile_pool(name="ag_sb_pool", bufs=8))
        ysb_pool = ctx.enter_context(tc.tile_pool(name="ysb_pool", bufs=3))

        last_evac = {}   # marker instructions, for anti-hoisting deps

        def emit_proj(x_dram, w_dram, sh, pidx, outs):
            """Feature-major projection (q/k) over cols [sh*1024, +1024)."""
            with tc.tile_pool(name="ps_proj", bufs=1, space="PSUM") as ps_proj:
                ps = [[ps_proj.tile([P, 512], FP32, name=f"pp{f}_{b}",
                                    tag=f"pp{f}_{b}")
                       for b in range(2)] for f in range(n_fc)]
                for kc in range(kc_n):
                    x_t = xw.tile([P, sh_w], compute_dt, name="x_t", tag="x")
                    nc.sync.dma_start(
                        out=x_t,
                        in_=x_dram[kc * P:(kc + 1) * P,
                                   sh * sh_w:(sh + 1) * sh_w])
                    w_t = xw.tile([P, fpc], compute_dt, name="w_t", tag="w")
                    nc.sync.dma_start(
                        out=w_t, in_=w_dram[kc * P:(kc + 1) * P, :])
                    for f in range(n_fc):
                        for b in range(2):
                            nc.tensor.matmul(
                                ps[f][b],
                                lhsT=w_t[:, f * P:(f + 1) * P],
                                rhs=x_t[:, b * 512:(b + 1) * 512],
                                start=(kc == 0), stop=(kc == kc_n - 1))
                for f in range(n_fc):
                    for b in range(2):
                        col = sh * sh_w + b * 512
                        act = nc.scalar.activation(
                            outs[f][:, col:col + 512], ps[f][b],
                            mybir.ActivationFunctionType.Identity,
                            bias=bias_sb[:, pidx * n_fc + f:
                                         pidx * n_fc + f + 1])
                        last_evac[(pidx, sh)] = act.ins

        def emit_proj_v(sh):
            """Natural-layout v projection: out[sq, f] via x-tile stationary.

            One [128, 512] psum tile per query tile; bias enters via a
            K=1 ones-row matmul that initializes the accumulator.
            """
            with tc.tile_pool(name="ps_vn", bufs=1, space="PSUM") as ps_vn:
                ps = [ps_vn.tile([P, fpc], FP32, name=f"pv_{st}",
                                 tag=f"pv_{st}") for st in range(8)]
                for st in range(8):
                    nc.tensor.matmul(ps[st], lhsT=ones_row, rhs=bvr_sb,
                                     start=True, stop=False)
                for kc in range(kc_n):
                    x_t = xw.tile([P, sh_w], compute_dt, name="x_t", tag="x")
                    nc.sync.dma_start(
                        out=x_t,
                        in_=vT[kc * P:(kc + 1) * P,
                               sh * sh_w:(sh + 1) * sh_w])
                    w_t = xw.tile([P, fpc], compute_dt, name="w_t", tag="w")
                    nc.sync.dma_start(
                        out=w_t, in_=wvT[kc * P:(kc + 1) * P, :])
                    for st in range(8):
                        nc.tensor.matmul(
                            ps[st],
                            lhsT=x_t[:, st * P:(st + 1) * P],
                            rhs=w_t,
                            start=False, stop=(kc == kc_n - 1))
                for st in range(8):
                    stg = sh * 8 + st
                    for h in range(hpc):
                        act = nc.scalar.activation(
                            vp[h][:, stg * VS:stg * VS + HD],
                            ps[st][:, h * HD:(h + 1) * HD],
                            mybir.ActivationFunctionType.Identity)
                        last_evac[(2, sh)] = act.ins

        def emit_attention_pair(jp, ps_sc, ps_pv0, ps_pv1):
            i0, i1 = 2 * jp, 2 * jp + 1
            for h in range(hpc):
                pv0 = ps_pv0.tile([P, HD + 1], FP32, name="pv0", tag="pv0")
                pv1 = ps_pv1.tile([P, HD + 1], FP32, name="pv1", tag="pv1")
                for m in range(jp + 1):
                    t0, t1 = 2 * m, 2 * m + 1
                    scW = ps_sc.tile([P, 4 * P], FP32, name="scW", tag="scW")
                    nc.tensor.matmul(
                        scW[:, 0:2 * P],
                        lhsT=kpT[h][:, t0 * P:(t0 + 1) * P],
                        rhs=qpT[h][:, i0 * P:(i0 + 2) * P],
                        start=True, stop=True)
                    nc.tensor.matmul(
                        scW[:, 2 * P:4 * P],
                        lhsT=kpT[h][:, t1 * P:(t1 + 1) * P],
                        rhs=qpT[h][:, i0 * P:(i0 + 2) * P],
                        start=True, stop=True)
                    if m == jp:  # final pair: [diag | 0 | full | diag]
                        nc.vector.tensor_add(scW, scW, mask_sb)
                    pTW = probs_pool.tile([P, 4 * P], compute_dt,
                                          name="pTW", tag="pTW")
                    nc.scalar.activation(
                        pTW, scW, mybir.ActivationFunctionType.Exp)
                    v0 = vp[h][:, t0 * VS:t0 * VS + HD + 1]
                    v1 = vp[h][:, t1 * VS:t1 * VS + HD + 1]
                    nc.tensor.matmul(pv0, lhsT=pTW[:, 0:P], rhs=v0,
                                     start=(m == 0), stop=False)
                    nc.tensor.matmul(pv0, lhsT=pTW[:, 2 * P:3 * P], rhs=v1,
                                     start=False, stop=(m == jp))
                    nc.tensor.matmul(pv1, lhsT=pTW[:, P:2 * P], rhs=v0,
                                     start=(m == 0), stop=False)
                    nc.tensor.matmul(pv1, lhsT=pTW[:, 3 * P:4 * P], rhs=v1,
                                     start=False, stop=(m == jp))
                for iq, pvx in ((i0, pv0), (i1, pv1)):
                    recip = small.tile([P, 1], FP32, name="recip", tag="recip")
                    nc.vector.reciprocal(recip, pvx[:, HD:HD + 1])
                    act = nc.scalar.activation(
                        attnN[h][:, iq * HD:(iq + 1) * HD], pvx[:, 0:HD],
                        mybir.ActivationFunctionType.Identity, scale=recip)
                    last_evac[("fin", jp)] = act.ins

        def emit_ship(cq, ps_tr):
            # transpose on the PE (it has slack during exp-paced bursts)
            w = chunk_ws[cq]
            col0 = cum[cq] - w
            for h in range(hpc):
                atT = attnT_pool.tile([P, w], compute_dt, name="atT",
                                      tag=f"atT{h}")
                for st in range(col0 // P, cum[cq] // P):
                    tr = ps_tr.tile([P, P], compute_dt, name="tr", tag="tr")
                    nc.tensor.transpose(
                        tr, attnN[h][:, st * HD:(st + 1) * HD], ident)
                    nc.vector.tensor_copy(
                        atT[:, (st - col0 // P) * P:(st - col0 // P + 1) * P],
                        tr)
                nc.sync.dma_start(
                    out=ag_in[cq][h * P:(h + 1) * P, :], in_=atT)
            nc.gpsimd.collective_compute(
                "AllGather", mybir.AluOpType.bypass,
                replica_groups=[list(range(ncores))],
                ins=[ag_in[cq][:, :]], outs=[ag_out[cq][:, :]])

        def emit_ag_load(cq, marker):
            w = chunk_ws[cq]
            ag_g = []
            for g in range(4):
                t = ag_sb_pool.tile([P, 8 * w], compute_dt,
                                    name="ag_sb", tag="agsb")
                dma = nc.gpsimd.dma_start(
                    out=t.rearrange("p (kc c) -> p kc c", kc=8),
                    in_=ag_out[cq][g * 8 * P:(g + 1) * 8 * P, :]
                    .rearrange("(kc p) c -> p kc c", p=P))
                add_dep_helper(dma.ins, marker, reason="agload pinned late")
                ag_g.append(t)
            return ag_g

        def emit_outproj(cq, ag_g, ps_y):
            w = chunk_ws[cq]
            col0 = cum[cq] - w
            for jm in range(n_fc):
                psy = ps_y.tile([P, w], FP32, name="psy", tag="psy")
                for kc in range(kc_n):
                    nc.tensor.matmul(
                        psy,
                        lhsT=wo_sb[kc][:, jm * P:(jm + 1) * P],
                        rhs=ag_g[kc // 8][:, (kc % 8) * w:(kc % 8 + 1) * w],
                        start=(kc == 0), stop=(kc == kc_n - 1))
                ysb = ysb_pool.tile([P, w], FP32, name="ysb", tag="ysb")
                nc.vector.tensor_copy(ysb, psy)
                nc.sync.dma_start(
                    out=yT[jm * P:(jm + 1) * P, col0:cum[cq]], in_=ysb)

        # ---------------- half 0: projections then attention burst 0 -----
        emit_proj(kT, wkT, 0, 1, kpT)
        emit_proj_v(0)
        emit_proj(qT, wqT, 0, 0, qpT)
        # burst-0 has no ps_y pool, so pv can double-buffer (8 banks total)
        with tc.tile_pool(name="ps_sc", bufs=2, space="PSUM") as ps_sc, \
             tc.tile_pool(name="ps_pv0", bufs=2, space="PSUM") as ps_pv0, \
             tc.tile_pool(name="ps_pv1", bufs=2, space="PSUM") as ps_pv1, \
             tc.tile_pool(name="ps_tr", bufs=2, space="PSUM") as ps_tr:
            for jp in range(4):
                emit_attention_pair(jp, ps_sc, ps_pv0, ps_pv1)
                if (2 * jp + 2) * P in cum:
                    emit_ship(cum.index((2 * jp + 2) * P), ps_tr)
        # prefetch out-proj weights on the scalar queue; pinned behind the
        # half-1 k-proj evacs so the scheduler cannot hoist them into
        # burst-0's exp stream
        wo_dmas = [nc.scalar.dma_start(
            out=wo_sb[kc], in_=woT[kc * P:(kc + 1) * P, :])
            for kc in range(kc_n)]

        # ---------------- half 1: projections then burst 1 + out-proj ----
        emit_proj(kT, wkT, 1, 1, kpT)
        for dma in wo_dmas:
            add_dep_helper(dma.ins, last_evac[(1, 1)],
                           reason="wo prefetch trails half-1 k evac")
        emit_proj_v(1)
        emit_proj(qT, wqT, 1, 0, qpT)
        # chunk 0/1 reloads: AGs 0/1 completed during half-1 projections.
        # Pinned behind the q evacs so their HBM traffic lands in burst-1's
        # DMA-quiet window instead of colliding with q's x/w streaming.
        ag_g0 = emit_ag_load(0, last_evac[(0, 1)])
        ag_g1 = emit_ag_load(1, last_evac[(0, 1)])
        with tc.tile_pool(name="ps_sc", bufs=2, space="PSUM") as ps_sc, \
             tc.tile_pool(name="ps_pv0", bufs=1, space="PSUM") as ps_pv0, \
             tc.tile_pool(name="ps_pv1", bufs=1, space="PSUM") as ps_pv1, \
             tc.tile_pool(name="ps_tr", bufs=2, space="PSUM") as ps_tr, \
             tc.tile_pool(name="ps_y", bufs=2, space="PSUM") as ps_y:
            # pair 7 first: its (small) AllGather is the natural tail, so
            # fire it as early as possible; out-proj chunks weave between
            # the remaining pairs to keep the PE warm while exp paces.
            # (Shipping the fat chunk c2 first instead measured ~22us
            # SLOWER on hardware — the long pair-7 overlaps the cc chain.)
            emit_attention_pair(7, ps_sc, ps_pv0, ps_pv1)
            emit_ship(4, ps_tr)
            emit_attention_pair(4, ps_sc, ps_pv0, ps_pv1)
            emit_attention_pair(5, ps_sc, ps_pv0, ps_pv1)
            emit_ship(2, ps_tr)
            emit_outproj(0, ag_g0, ps_y)
            emit_attention_pair(6, ps_sc, ps_pv0, ps_pv1)
            emit_ship(3, ps_tr)
            emit_outproj(1, ag_g1, ps_y)
            ag_g4 = emit_ag_load(4, last_evac[("fin", 5)])
            emit_outproj(4, ag_g4, ps_y)
            ag_g2 = emit_ag_load(2, last_evac[("fin", 6)])
            emit_outproj(2, ag_g2, ps_y)
            ag_g3 = emit_ag_load(3, last_evac[("fin", 6)])
            emit_outproj(3, ag_g3, ps_y)
    nc.compile()
    return nc


# ------------------------------------------------------------- host side --

def _rope_fold(W, bvec, n_heads, scale):
    """Fold head-indexed RoPE rotation (and scale) into projection weights."""
    inv = 1.0 / (ROPE_BASE ** (np.arange(0, HD, 2, dtype=np.float32) / HD))
    ang = np.arange(n_heads, dtype=np.float32)[:, None] * inv[None, :]
    cos = np.cos(ang)[:, :, None]   # [H, HD/2, 1]
    sin = np.sin(ang)[:, :, None]
    Wr = W.reshape(n_heads, HD // 2, 2, -1).astype(np.float32)
    w0, w1 = Wr[:, :, 0, :], Wr[:, :, 1, :]
    out = np.empty_like(Wr)
    out[:, :, 0, :] = cos * w0 - sin * w1
    out[:, :, 1, :] = sin * w0 + cos * w1
    Wf = out.reshape(W.shape) * scale
    br = bvec.reshape(n_heads, HD // 2, 2).astype(np.float32)
    cos2, sin2 = cos[:, :, 0], sin[:, :, 0]
    bout = np.empty_like(br)
    bout[:, :, 0] = cos2 * br[:, :, 0] - sin2 * br[:, :, 1]
    bout[:, :, 1] = sin2 * br[:, :, 0] + cos2 * br[:, :, 1]
    bf = bout.reshape(bvec.shape) * scale
    return Wf, bf


def _make_maskw():
    # wide mask for the final key-tile pair of scW = [sk0, sk1] x [sq0, sq1]:
    # blocks [diag | zeros | full | diag]; valid iff sk <= sq (transposed).
    r = np.arange(P, dtype=np.int64)[:, None]
    c = np.arange(P, dtype=np.int64)[None, :]
    diag = np.where(r <= c, 0.0, -1e30).astype(np.float32)
    zero = np.zeros((P, P), np.float32)
    full = np.full((P, P), -1e30, np.float32)
    return np.concatenate([diag, zero, full, diag], axis=1)  # [128, 512]


def _bf16(x):
    return np.ascontiguousarray(np.asarray(x, dtype=np.float32)).astype(
        ml_dtypes.bfloat16)


_NC_CACHE = {}


def _get_nc():
    if "nc" not in _NC_CACHE:
        _NC_CACHE["nc"] = build_nc()
    return _NC_CACHE["nc"]


def prepare_in_maps(q, k, v, Wq, bq, Wk, bk, Wv, bv, Wo, bo):
    q = np.asarray(q, np.float32)
    k = np.asarray(k, np.float32)
    v = np.asarray(v, np.float32)
    Wq = np.asarray(Wq, np.float32)
    Wk = np.asarray(Wk, np.float32)
    Wv = np.asarray(Wv, np.float32)
    Wo = np.asarray(Wo, np.float32)
    bq = np.asarray(bq, np.float32)
    bk = np.asarray(bk, np.float32)
    bv = np.asarray(bv, np.float32)

    scale = 1.0 / math.sqrt(HD)
    Wqf, bqf = _rope_fold(Wq, bq, H, scale)
    Wkf, bkf = _rope_fold(Wk, bk, H, 1.0)

    qT = _bf16(q[0].T)
    kT = _bf16(k[0].T)
    vT = _bf16(v[0].T)
    maskw = _make_maskw()

    in_maps = []
    for c in range(NCORES):
        sl = slice(c * FPC, (c + 1) * FPC)
        bias = np.stack(
            [bqf[sl].reshape(4, P)[f] for f in range(4)]
            + [bkf[sl].reshape(4, P)[f] for f in range(4)]
            + [bv[sl].reshape(4, P)[f] for f in range(4)], axis=1
        ).astype(np.float32)  # [128, 12]
        in_maps.append({
            "qT": qT, "kT": kT, "vT": vT,
            "wqT": _bf16(Wqf[sl].T), "wkT": _bf16(Wkf[sl].T),
            "wvT": _bf16(Wv[sl].T), "woT": _bf16(Wo[sl].T),
            "bqkv": np.ascontiguousarray(bias),
            "bvr": _bf16(bv[sl][None, :]),
            "maskw": maskw,
        })
    return in_maps


def postprocess(results, bo):
    bo = np.asarray(bo, np.float32)
    out = np.concatenate(
        [np.asarray(results[c]["yT"], np.float32).T
         for c in range(NCORES)], axis=1)
    out = out + bo[None, :]
    return out[None].astype(np.float32)


def kernel(q, k, v, Wq, bq, Wk, bk, Wv, bv, Wo, bo):
    in_maps = prepare_in_maps(q, k, v, Wq, bq, Wk, bk, Wv, bv, Wo, bo)
    nc = _get_nc()
    res = bass_utils.run_bass_kernel_spmd(
        nc, in_maps, core_ids=list(range(NCORES)))
    return postprocess(res.results, bo)
